# revision 1
# baseline (speedup 1.0000x reference)
"""Trainium2 Bass kernel for a dense transformer block.

reference: x -> LN1 -> 16-head causal attention (+residual) -> LN2 -> MLP
(+residual), x: [2, 2048, 1024] fp32.

Sharding: sequence-parallel with zigzag load balancing, zero collectives.
Core c (of 8) handles batch c//4 and query chunks j=c%4 and 7-j (256 rows
each => 512 rows/core). Each core recomputes LN1+K/V for its batch's first
1792 rows (the longest strict prefix any chunk needs); the 512 own rows'
K/V/Q come from a separate fixed-position path so one uniform program runs
on all 8 cores (SPMD), with per-core behavior carried entirely by input
data (augmented score rows + additive masks).

Matmul layout trick: activations are kept feature-major (transposed via PE)
so every matmul's contraction dim sits on partitions; softmax denominators
ride the AV matmul as an appended ones-column of V; per-token divides use
gpsimd partition_broadcast. Matmuls run in float32r (TF32-like, 1 cyc/row
at moving-dim >= 256; stationary free dim must be 64 or 128; producers
must write float32r).
"""

import sys

sys.path.insert(0, "/opt/trn_rl_repo")

from contextlib import ExitStack

import numpy as np

import concourse.bacc as bacc
import concourse.mybir as mybir
import concourse.tile as tile
from concourse.bass_utils import run_bass_kernel_spmd
from concourse.masks import make_identity

F32 = mybir.dt.float32
MM = mybir.dt.float32r  # matmul operand dtype
AF = mybir.ActivationFunctionType
ALU = mybir.AluOpType

B, P, D, H, DH = 2, 2048, 1024, 16, 64
FF = 4 * D
EPS = 1e-5
NCORES = 8
KV = 1792            # rect-path kv rows (longest strict prefix = 7*256)
KC = KV // 128       # 14 rect kv chunks
QL = 512             # query rows per core
DC = D // 128        # 8 contraction chunks over D
FC = FF // 128       # 32 f-chunks
BIG = 30000.0        # additive mask magnitude; exp(-30000) == 0 in fp32

# N-tiling of the [*, KV] projection outputs (PSUM bank is 512 fp32 wide)
NT = [(0, 512), (512, 512), (1024, 512), (1536, 256)]


def _ln_tile(nc, spool, eps_t, src, dst):
    """dst = (src - mean(src)) * rsqrt(var(src) + EPS), rows on partitions."""
    stats = spool.tile([128, 2, nc.vector.BN_STATS_DIM], F32, tag="ln_stats")
    for sg in range(2):
        nc.vector.bn_stats(out=stats[:, sg, :], in_=src[:, sg * 512:(sg + 1) * 512])
    mv = spool.tile([128, nc.vector.BN_AGGR_DIM], F32, tag="ln_mv")
    nc.vector.bn_aggr(out=mv[:], in_=stats[:])
    rstd = spool.tile([128, 1], F32, tag="ln_rstd")
    nc.scalar.activation(out=rstd[:], in_=mv[:, 1:2], func=AF.Sqrt, bias=eps_t[:])
    nc.vector.reciprocal(out=rstd[:], in_=rstd[:])
    nc.vector.tensor_scalar(out=dst, in0=src, scalar1=mv[:, 0:1], scalar2=rstd[:],
                            op0=ALU.subtract, op1=ALU.mult)


def build_nc():
    nc = bacc.Bacc(trn_type="TRN2")

    xb = nc.declare_dram_parameter("xb", [KV, D], F32, isOutput=False)
    xq = nc.declare_dram_parameter("xq", [QL, D], F32, isOutput=False)
    wq = nc.declare_dram_parameter("wq", [D, D], MM, isOutput=False)
    wk = nc.declare_dram_parameter("wk", [D, D], MM, isOutput=False)
    wv = nc.declare_dram_parameter("wv", [D, D], MM, isOutput=False)
    bqkv = nc.declare_dram_parameter("bqkv", [3, D], F32, isOutput=False)
    wp = nc.declare_dram_parameter("wp", [D, D], MM, isOutput=False)
    w1 = nc.declare_dram_parameter("w1", [D, FF], MM, isOutput=False)
    w2 = nc.declare_dram_parameter("w2", [FF, D], MM, isOutput=False)
    b1v = nc.declare_dram_parameter("b1v", [FF], F32, isOutput=False)
    bpv = nc.declare_dram_parameter("bpv", [D], F32, isOutput=False)
    b2v = nc.declare_dram_parameter("b2v", [D], F32, isOutput=False)
    augq = nc.declare_dram_parameter("augq", [2, QL], MM, isOutput=False)
    augk = nc.declare_dram_parameter("augk", [2, KV], MM, isOutput=False)
    dmask = nc.declare_dram_parameter("dmask", [4, 128, 256], F32, isOutput=False)
    out = nc.declare_dram_parameter("out", [QL, D], F32, isOutput=True)

    # DRAM views reshaped for partition-major DMA
    wq_v = wq.ap().rearrange("(dc p) e -> p dc e", p=128)
    wk_v = wk.ap().rearrange("(dc p) e -> p dc e", p=128)
    wv_v = wv.ap().rearrange("(dc p) e -> p dc e", p=128)
    w1_v = w1.ap().rearrange("(dc p) f -> p dc f", p=128)
    bqkv_v = bqkv.ap().rearrange("b (h e) -> e b h", e=DH)      # [64, 3, 16]
    b1_v = b1v.ap().rearrange("(fc p) -> p fc", p=128)          # [128, 32]
    bp_v = bpv.ap().rearrange("(a d) -> a d", a=1)
    b2_v = b2v.ap().rearrange("(a d) -> a d", a=1)
    dm_v = dmask.ap().rearrange("c p n -> p c n")

    with tile.TileContext(nc) as tc, ExitStack() as ctx:
        persist = ctx.enter_context(tc.tile_pool(name="persist", bufs=1))
        trps = ctx.enter_context(tc.tile_pool(name="trps", bufs=2, space="PSUM"))
        spool = ctx.enter_context(tc.tile_pool(name="spool", bufs=3))

        # ---- constants
        ident = persist.tile([128, 128], F32)
        make_identity(nc, ident[:])
        eps_t = persist.tile([128, 1], F32)
        nc.vector.memset(eps_t[:], EPS)
        dm = persist.tile([128, 4, 256], F32)
        nc.sync.dma_start(dm[:], dm_v)
        bqkv_sb = persist.tile([64, 3, H], F32)
        nc.sync.dma_start(bqkv_sb[:], bqkv_v)
        b1_sb = persist.tile([128, FC], F32)
        nc.sync.dma_start(b1_sb[:], b1_v)
        bp_row = persist.tile([1, D], F32)
        nc.sync.dma_start(bp_row[:], bp_v)
        bp_bc = persist.tile([128, D], F32)
        nc.gpsimd.partition_broadcast(bp_bc[:], bp_row[0:1, :])
        b2_row = persist.tile([1, D], F32)
        nc.sync.dma_start(b2_row[:], b2_v)
        b2_bc = persist.tile([128, D], F32)
        nc.gpsimd.partition_broadcast(b2_bc[:], b2_row[0:1, :])

        oT = persist.tile([128, DC, QL], MM)   # attention out, feature-major

        def transpose_to(src, dst, pn=128):
            """dst[cols, rows] = src[rows, cols].T via PE (src [pn,128])."""
            tp = trps.tile([128, 128], F32, tag="tr")
            nc.tensor.transpose(tp[:, 0:pn], src, ident[0:pn, 0:pn])
            nc.vector.tensor_copy(dst, tp[:, 0:pn])

        with tc.tile_pool(name="ph2big", bufs=1) as ph2big:
            hT = ph2big.tile([128, DC, KV], MM)    # LN1(x_b[:1792]) transposed
            hqT = ph2big.tile([128, DC, QL], MM)   # LN1(x_own) transposed

            # ===== Phase 1: LN1 -> transposed activations =====
            with tc.tile_pool(name="xpool", bufs=3) as xpool, \
                 tc.tile_pool(name="hpool", bufs=2) as hpool:
                for i in range(KC):
                    xt = xpool.tile([128, D], F32, tag="xt")
                    nc.sync.dma_start(xt[:], xb.ap()[128 * i:128 * (i + 1), :])
                    ht = hpool.tile([128, D], F32, tag="ht")
                    _ln_tile(nc, spool, eps_t, xt[:], ht[:])
                    for dc in range(DC):
                        transpose_to(ht[:, 128 * dc:128 * (dc + 1)],
                                     hT[:, dc, 128 * i:128 * (i + 1)])
                for i in range(4):
                    xt = xpool.tile([128, D], F32, tag="xt")
                    nc.sync.dma_start(xt[:], xq.ap()[128 * i:128 * (i + 1), :])
                    ht = hpool.tile([128, D], F32, tag="ht")
                    _ln_tile(nc, spool, eps_t, xt[:], ht[:])
                    for dc in range(DC):
                        transpose_to(ht[:, 128 * dc:128 * (dc + 1)],
                                     hqT[:, dc, 128 * i:128 * (i + 1)])

            # ===== Phase 2: per-head QKV + attention =====
            with tc.tile_pool(name="wpool", bufs=2) as wpool, \
                 tc.tile_pool(name="kqpool", bufs=2) as kqpool, \
                 tc.tile_pool(name="vpool", bufs=2) as vpool, \
                 tc.tile_pool(name="apool", bufs=3) as apool, \
                 tc.tile_pool(name="rpool", bufs=2) as rpool, \
                 tc.tile_pool(name="qkvps", bufs=2, space="PSUM") as qkvps, \
                 tc.tile_pool(name="spsum", bufs=2, space="PSUM") as spsum, \
                 tc.tile_pool(name="opsum", bufs=1, space="PSUM") as opsum:

                for h in range(H):
                    wq_t = wpool.tile([128, DC, DH], MM, tag="wq_t")
                    nc.sync.dma_start(wq_t[:], wq_v[:, :, DH * h:DH * (h + 1)])
                    wk_t = wpool.tile([128, DC, DH], MM, tag="wk_t")
                    nc.sync.dma_start(wk_t[:], wk_v[:, :, DH * h:DH * (h + 1)])
                    wv_t = wpool.tile([128, DC, DH], MM, tag="wv_t")
                    nc.sync.dma_start(wv_t[:], wv_v[:, :, DH * h:DH * (h + 1)])

                    def proj(w_t, rhs_sb, n0, nl, ps_tag="qkv"):
                        ps = qkvps.tile([64, 512], F32, tag=ps_tag)
                        for dc in range(DC):
                            nc.tensor.matmul(ps[:, 0:nl], w_t[:, dc, :],
                                             rhs_sb[:, dc, n0:n0 + nl],
                                             start=(dc == 0), stop=(dc == DC - 1))
                        return ps

                    # q' [66, 512]: rows 0-63 q, rows 64-65 aug block indicators
                    qp = kqpool.tile([66, QL], MM, tag="qp")
                    ps = proj(wq_t, hqT, 0, QL)
                    nc.vector.tensor_scalar_add(qp[0:64, :], ps[:],
                                                bqkv_sb[:, 0, h:h + 1])
                    nc.sync.dma_start(qp[64:66, :], augq.ap())

                    # k' [66, 1792]: rows 64-65 aug prefix-validity rows
                    kp = kqpool.tile([66, KV], MM, tag="kp")
                    for n0, nl in NT:
                        ps = proj(wk_t, hT, n0, nl)
                        nc.vector.tensor_scalar_add(kp[0:64, n0:n0 + nl],
                                                    ps[:, 0:nl],
                                                    bqkv_sb[:, 1, h:h + 1])
                    nc.sync.dma_start(kp[64:66, :], augk.ap())

                    # v: project feature-major with an appended ones row
                    # (becomes col 64 = softmax denominator after transpose),
                    # retranspose chunks into [128, kc, 128] (cols 65+ zero).
                    vt = vpool.tile([65, KV], F32, tag="vt")
                    for n0, nl in NT:
                        ps = proj(wv_t, hT, n0, nl)
                        nc.vector.tensor_scalar_add(vt[0:64, n0:n0 + nl], ps[:, 0:nl],
                                                    bqkv_sb[:, 2, h:h + 1])
                    nc.vector.memset(vt[64:65, :], 1.0)
                    vp = vpool.tile([128, KC, 128], MM, tag="vp")
                    nc.vector.memset(vp[:, :, :].bitcast(F32), 0.0)
                    for kc in range(KC):
                        transpose_to(vt[:, 128 * kc:128 * (kc + 1)],
                                     vp[:, kc, 0:65], pn=65)

                    # diag-path k,v (own rows) from hqT
                    kd = kqpool.tile([64, QL], MM, tag="kd")
                    ps = proj(wk_t, hqT, 0, QL)
                    nc.vector.tensor_scalar_add(kd[:], ps[:], bqkv_sb[:, 1, h:h + 1])
                    vd = vpool.tile([65, QL], F32, tag="vd")
                    ps = proj(wv_t, hqT, 0, QL)
                    nc.vector.tensor_scalar_add(vd[0:64, :], ps[:], bqkv_sb[:, 2, h:h + 1])
                    nc.vector.memset(vd[64:65, :], 1.0)
                    vpd = vpool.tile([128, 4, 128], MM, tag="vpd")
                    nc.vector.memset(vpd[:, :, :].bitcast(F32), 0.0)
                    for ci in range(4):
                        transpose_to(vd[:, 128 * ci:128 * (ci + 1)],
                                     vpd[:, ci, 0:65], pn=65)

                    # ---- attention
                    # Two independent accumulators, one per query block's
                    # 256 columns (no PSUM subrange accumulation). Rect chunks
                    # >= 6 can never be valid for block A on any core
                    # (A-validity needs kc < 2j <= 6), so they only feed ops_b.
                    ops_a = opsum.tile([128, 256], F32, tag="ops_a")
                    ops_b = opsum.tile([128, 256], F32, tag="ops_b")
                    for kc in range(KC):
                        nw = QL if kc < 6 else 256
                        qo = 0 if kc < 6 else 256
                        sps = spsum.tile([128, nw], F32, tag="sps")
                        nc.tensor.matmul(sps[:], kp[:, 128 * kc:128 * (kc + 1)],
                                         qp[:, qo:qo + nw], start=True, stop=True)
                        att = apool.tile([128, nw], MM, tag="att")
                        nc.scalar.activation(att[:], sps[:], AF.Exp)
                        if kc < 6:
                            nc.tensor.matmul(ops_a[:], vp[:, kc, :],
                                             att[:, 0:256],
                                             start=(kc == 0), stop=False)
                            nc.tensor.matmul(ops_b[:], vp[:, kc, :],
                                             att[:, 256:512],
                                             start=(kc == 0), stop=False)
                        else:
                            nc.tensor.matmul(ops_b[:], vp[:, kc, :], att[:],
                                             start=False, stop=False)
                    for ci in range(4):
                        qoff = 256 * (ci // 2)
                        sps = spsum.tile([128, 256], F32, tag="sps")
                        nc.tensor.matmul(sps[:], kd[:, 128 * ci:128 * (ci + 1)],
                                         qp[0:64, qoff:qoff + 256],
                                         start=True, stop=True)
                        nc.vector.tensor_add(sps[:], sps[:], dm[:, ci, :])
                        att = apool.tile([128, 256], MM, tag="att")
                        nc.scalar.activation(att[:], sps[:], AF.Exp)
                        tgt = ops_a if ci < 2 else ops_b
                        nc.tensor.matmul(tgt[:], vpd[:, ci, :], att[:],
                                         start=False, stop=(ci in (1, 3)))

                    # normalize: o/s (s = ops row 64); odd heads land at
                    # partition 64 of oT, moved there by SBUF->SBUF DMA.
                    rec = rpool.tile([1, QL], F32, tag="rec")
                    nc.vector.reciprocal(rec[0:1, 0:256], ops_a[64:65, :])
                    nc.vector.reciprocal(rec[0:1, 256:512], ops_b[64:65, :])
                    sbc = rpool.tile([64, QL], F32, tag="sbc")
                    nc.gpsimd.partition_broadcast(sbc[:], rec[0:1, :])
                    prow = (h % 2) * 64
                    nc.vector.tensor_mul(oT[prow:prow + 64, h // 2, 0:256],
                                         ops_a[0:64, :], sbc[:, 0:256])
                    nc.vector.tensor_mul(oT[prow:prow + 64, h // 2, 256:512],
                                         ops_b[0:64, :], sbc[:, 256:512])

        # ===== Phase 3: attn_out = oT.T @ Wp; xmid = attn_out + xq + bp =====
        with tc.tile_pool(name="ph3big", bufs=1) as ph3big:
            xmid = ph3big.tile([128, 4, D], F32)
            with tc.tile_pool(name="wps", bufs=2) as wps, \
                 tc.tile_pool(name="xqp", bufs=2) as xqp, \
                 tc.tile_pool(name="finps", bufs=1, space="PSUM") as finps:
                for dh in range(2):
                    pss = [finps.tile([128, 512], F32, tag=f"fin{t}",
                                      name=f"fin{t}_{dh}")
                           for t in range(4)]
                    for dc in range(DC):
                        wpt = wps.tile([128, 512], MM, tag="wpt")
                        nc.sync.dma_start(
                            wpt[:],
                            wp.ap()[128 * dc:128 * (dc + 1), 512 * dh:512 * (dh + 1)])
                        for t in range(4):
                            nc.tensor.matmul(pss[t][:],
                                             oT[:, dc, 128 * t:128 * (t + 1)],
                                             wpt[:], start=(dc == 0),
                                             stop=(dc == DC - 1))
                    for t in range(4):
                        xqt = xqp.tile([128, 512], F32, tag="xqt")
                        nc.sync.dma_start(
                            xqt[:],
                            xq.ap()[128 * t:128 * (t + 1), 512 * dh:512 * (dh + 1)])
                        sl = xmid[:, t, 512 * dh:512 * (dh + 1)]
                        nc.vector.tensor_add(sl, pss[t][:], xqt[:])
                        nc.vector.tensor_add(sl, sl, bp_bc[:, 512 * dh:512 * (dh + 1)])

            # ===== Phase 4: LN2 -> h2T =====
            with tc.tile_pool(name="ph5big", bufs=1) as ph5big:
                h2T = ph5big.tile([128, DC, QL], MM)
                mT = ph5big.tile([128, FC, QL], MM)
                with tc.tile_pool(name="hpool2", bufs=2) as hpool2:
                    for i in range(4):
                        ht = hpool2.tile([128, D], F32, tag="h2t")
                        _ln_tile(nc, spool, eps_t, xmid[:, i, :], ht[:])
                        for dc in range(DC):
                            transpose_to(ht[:, 128 * dc:128 * (dc + 1)],
                                         h2T[:, dc, 128 * i:128 * (i + 1)])

                # ===== Phase 5: MLP + residual + output =====
                with tc.tile_pool(name="w1p", bufs=2) as w1p, \
                     tc.tile_pool(name="w2p", bufs=2) as w2p, \
                     tc.tile_pool(name="opool", bufs=3) as opool, \
                     tc.tile_pool(name="finps2", bufs=1, space="PSUM") as finps2, \
                     tc.tile_pool(name="mps", bufs=2, space="PSUM") as mps:
                    for dh in range(2):
                        pss = [finps2.tile([128, 512], F32, tag=f"fo{t}",
                                           name=f"fo{t}_{dh}")
                               for t in range(4)]
                        for fc in range(FC):
                            if dh == 0:
                                w1t = w1p.tile([128, DC, 128], MM, tag="w1t")
                                nc.sync.dma_start(
                                    w1t[:], w1_v[:, :, 128 * fc:128 * (fc + 1)])
                                mp = mps.tile([128, QL], F32, tag="mp")
                                for dc in range(DC):
                                    nc.tensor.matmul(mp[:], w1t[:, dc, :],
                                                     h2T[:, dc, :],
                                                     start=(dc == 0),
                                                     stop=(dc == DC - 1))
                                nc.scalar.activation(mT[:, fc, :], mp[:], AF.Gelu,
                                                     bias=b1_sb[:, fc:fc + 1])
                            w2t = w2p.tile([128, 512], MM, tag="w2t")
                            nc.sync.dma_start(
                                w2t[:],
                                w2.ap()[128 * fc:128 * (fc + 1),
                                        512 * dh:512 * (dh + 1)])
                            for t in range(4):
                                nc.tensor.matmul(pss[t][:],
                                                 mT[:, fc, 128 * t:128 * (t + 1)],
                                                 w2t[:], start=(fc == 0),
                                                 stop=(fc == FC - 1))
                        for t in range(4):
                            ot = opool.tile([128, 512], F32, tag="ot")
                            nc.vector.tensor_add(ot[:], pss[t][:],
                                                 xmid[:, t, 512 * dh:512 * (dh + 1)])
                            nc.vector.tensor_add(ot[:], ot[:],
                                                 b2_bc[:, 512 * dh:512 * (dh + 1)])
                            nc.sync.dma_start(
                                out.ap()[128 * t:128 * (t + 1),
                                         512 * dh:512 * (dh + 1)],
                                ot[:])

    nc.compile()
    return nc


_NC_CACHE = {}


def _get_nc():
    if "nc" not in _NC_CACHE:
        _NC_CACHE["nc"] = build_nc()
    return _NC_CACHE["nc"]


def _host_pack(inputs):
    x = np.ascontiguousarray(np.asarray(inputs["x"], dtype=np.float32))
    Wq = np.asarray(inputs["Wq"], np.float32).transpose(1, 0, 2).reshape(D, D)
    Wk = np.asarray(inputs["Wk"], np.float32).transpose(1, 0, 2).reshape(D, D)
    Wv = np.asarray(inputs["Wv"], np.float32).transpose(1, 0, 2).reshape(D, D)
    Wp = np.asarray(inputs["Wp"], np.float32)
    bp = np.asarray(inputs["bp"], np.float32)
    W1 = np.asarray(inputs["W1"], np.float32)
    b1 = np.asarray(inputs["b1"], np.float32)
    W2 = np.asarray(inputs["W2"], np.float32)
    b2 = np.asarray(inputs["b2"], np.float32)
    g1 = np.asarray(inputs["g1"], np.float32)
    be1 = np.asarray(inputs["be1"], np.float32)
    g2 = np.asarray(inputs["g2"], np.float32)
    be2 = np.asarray(inputs["be2"], np.float32)

    scale = np.float32(np.float64(D) ** -0.5)  # 1/32, exact power of two
    wq_p = np.ascontiguousarray(Wq * g1[:, None] * scale)
    wk_p = np.ascontiguousarray(Wk * g1[:, None])
    wv_p = np.ascontiguousarray(Wv * g1[:, None])
    # biases induced by the LN shift (be1), folded into q/k/v
    bqkv = np.stack([be1 @ Wq * scale, be1 @ Wk, be1 @ Wv]).astype(np.float32)
    w1_p = np.ascontiguousarray(W1 * g2[:, None])
    b1_p = (b1 + be2 @ W1).astype(np.float32)

    augq = np.zeros((2, QL), np.float32)
    augq[0, 0:256] = 1.0
    augq[1, 256:512] = 1.0

    # diag masks: additive, 0 keep / -BIG drop. Diag kv rows = own 512 rows
    # (A chunk then B chunk); columns = own 512 queries (A then B).
    dmask = np.empty((4, 128, 256), np.float32)
    ii = np.arange(128)
    jj = np.arange(256)
    for ci in range(4):
        loc = 128 * (ci % 2) + ii[:, None]          # row pos within the block
        keep = loc <= jj[None, :]                   # causal within block
        dmask[ci] = np.where(keep, 0.0, -BIG)

    shared = dict(wq=wq_p, wk=wk_p, wv=wv_p, bqkv=bqkv, wp=Wp, w1=w1_p,
                  b1v=b1_p, w2=W2, bpv=bp, b2v=b2, augq=augq,
                  dmask=np.ascontiguousarray(dmask))

    in_maps = []
    for c in range(NCORES):
        b, j = c // 4, c % 4
        xb_c = np.ascontiguousarray(x[b, :KV])
        xq_c = np.ascontiguousarray(
            np.concatenate([x[b, 256 * j:256 * (j + 1)],
                            x[b, 256 * (7 - j):256 * (8 - j)]], axis=0))
        augk = np.zeros((2, KV), np.float32)
        augk[0, 256 * j:] = -BIG        # block A valid rect prefix: t < 256j
        augk[1, 256 * (7 - j):] = -BIG  # block B valid rect prefix: t < 256(7-j)
        in_maps.append(dict(shared, xb=xb_c, xq=xq_c, augk=augk))
    return x, in_maps


def _unshard(results):
    out = np.empty((B, P, D), np.float32)
    for c in range(NCORES):
        b, j = c // 4, c % 4
        o = results[c]["out"]
        out[b, 256 * j:256 * (j + 1)] = o[0:256]
        out[b, 256 * (7 - j):256 * (8 - j)] = o[256:512]
    return out


def kernel(**inputs):
    x, in_maps = _host_pack(inputs)
    nc = _get_nc()
    res = run_bass_kernel_spmd(nc, in_maps, core_ids=list(range(NCORES)))
    return _unshard(res.results)

